# revision 2
# baseline (speedup 1.0000x reference)
"""Trainium2 Bass kernel for nn_HardQuadTripletSOSRLoss.

Sharding: 8 cores = 2 batches x 4 HW-shards (4096 grid cells each).
Device job (per core): scores = fp8(kp1_desc[b]) @ fp8(desc2f[b, shard])
via DoubleRow fp8 matmuls -> fp32 PSUM, 8 units of [128 rows, 2048 cells]
(unit u = (row-tile t, window w), PSUM half u%2).  Each unit is consumed by
exactly one engine (concurrent PSUM reads by two engines are illegal):
  - w=0 'D' lane: DVE tensor_reduce(max) over contiguous groups of 8 cells
    -> [128, 256] bf16 group maxima.
  - w=1 'A' lane: ACT copies the raw unit -> [128, 2048] bf16.
All candidate data is DMA'd to the host, which max-reduces the raw windows
to the same G=8 granularity, takes the top-K groups per row, rescores those
cells exactly in fp32, drops masked (neighbor) cells, and applies a
certificate: rows where hidden cells could reach the top-16 are recomputed
exactly.  All other stages (sampling, geometry, masks, SOS, loss) run on
host, vectorized."""

import numpy as np
import ml_dtypes

import concourse.bass as bass
import concourse.mybir as mybir
from concourse import bacc
from concourse.bass_utils import run_bass_kernel_spmd

# ---- problem constants (hardcoded per contract) ----
B, N, C, H, W = 2, 512, 128, 128, 128
HW = H * W
GS = 8
NUM_NEG = 16
SOS_NEG = 8
MARGIN = 1.0
NSHARD = 4
SHW = HW // NSHARD          # 4096 cells per shard
WIN = 2048                  # cells per unit
RT = N // 128               # 4 row tiles
CPB = 512                   # columns per matmul (one PSUM bank)
GRP = 8                     # cells per group (contiguous)
NGRP = WIN // GRP           # 256 groups per unit
TCOLS = NGRP + WIN          # candidate cols per row-tile: 256 reduced + 2048 raw

KSEL = 48                   # top-K groups rescored exactly per row
DELTA = 0.22                # certificate margin (fp8 dot + bf16 rounding)

F32 = mybir.dt.float32
BF16 = mybir.dt.bfloat16
F8 = mybir.dt.float8e4
BF = ml_dtypes.bfloat16
F8NP = ml_dtypes.float8_e4m3fn

_NC_CACHE = {}
LAST_RESULTS = None  # BassKernelResults of most recent device run (for test.py)


def _build_nc():
    nc = bacc.Bacc("TRN2", target_bir_lowering=False, debug=False, num_devices=8)

    # lhsT (p, t, i, n'): q[t*128+n', p+64i];  rhs (p, i, m): d[p+64i, m]
    lhsT = nc.dram_tensor("lhsT", [64, RT, 2, 128], F8, kind="ExternalInput")
    rhs = nc.dram_tensor("rhs", [64, 2, SHW], F8, kind="ExternalInput")
    # per row-tile: 256 bf16 group maxima (w=0) + 2048 bf16 raw (w=1)
    cand = nc.dram_tensor("cand", [128, RT * TCOLS], BF16, kind="ExternalOutput")

    with (
        nc.sbuf_tensor([64, RT, 2, 128], F8) as lhsT_sb,
        nc.sbuf_tensor([64, 2, SHW], F8) as rhs_sb,
        nc.sbuf_tensor([128, RT, NGRP], BF16) as red_sb,
        nc.sbuf_tensor([128, RT, WIN], BF16) as cp_sb,
        nc.psum_tensor([128, 2 * WIN], F32) as ps,
        nc.semaphore() as dm_l,
        nc.semaphore() as dm_h0,
        nc.semaphore() as dm_h1,
        nc.semaphore() as mm_sem,
        nc.semaphore() as dve_sem,
        nc.semaphore() as act_sem,
        nc.semaphore() as out_sem,
        nc.Block() as block,
    ):

        @block.sync
        def _(sync):
            sync.dma_start(lhsT_sb[:], lhsT[:]).then_inc(dm_l, 16)
            sync.dma_start(
                rhs_sb[:, :, :WIN], rhs[:, :, :WIN]
            ).then_inc(dm_h0, 16)
            # raw (A) chunks for t=1,3 on the sync queue
            for t in (1, 3):
                sync.wait_ge(act_sem, t + 1)
                sync.dma_start(
                    cand[:, t * TCOLS + NGRP : (t + 1) * TCOLS], cp_sb[:, t, :]
                ).then_inc(out_sem, 16)
            sync.wait_ge(out_sem, 96)

        @block.gpsimd
        def _(gpsimd):
            gpsimd.dma_start(
                rhs_sb[:, :, WIN:], rhs[:, :, WIN:]
            ).then_inc(dm_h1, 16)
            # raw (A) chunks for t=0,2 + all reduced (D) chunks
            for t in (0, 2):
                gpsimd.wait_ge(act_sem, t + 1)
                gpsimd.dma_start(
                    cand[:, t * TCOLS + NGRP : (t + 1) * TCOLS], cp_sb[:, t, :]
                ).then_inc(out_sem, 16)
            gpsimd.wait_ge(dve_sem, RT)
            for t in range(RT):
                gpsimd.dma_start(
                    cand[:, t * TCOLS : t * TCOLS + NGRP], red_sb[:, t, :]
                ).then_inc(out_sem, 16)

        @block.tensor
        def _(tensor):
            for u in range(2 * RT):
                t, w = u // 2, u % 2
                if u >= 2:
                    # PSUM half w freed once unit u-2 was consumed
                    if (u - 2) % 2 == 0:
                        tensor.wait_ge(dve_sem, (u - 2) // 2 + 1)
                    else:
                        tensor.wait_ge(act_sem, (u - 2) // 2 + 1)
                pso = w * WIN
                for c in range(WIN // CPB):
                    if u == 0 and c == 0:
                        tensor.wait_ge(dm_l, 16)
                        tensor.wait_ge(dm_h0, 16)
                    elif u == 1 and c == 0:
                        tensor.wait_ge(dm_h1, 16)
                    col = w * WIN + c * CPB
                    mm = nc.tensor.matmul(
                        ps[:, pso + c * CPB : pso + (c + 1) * CPB],
                        lhsT_sb[:, t, :, :],
                        rhs_sb[:, :, col : col + CPB],
                        start=True,
                        stop=True,
                        perf_mode=mybir.MatmulPerfMode.DoubleRow,
                    )
                    if c == WIN // CPB - 1:
                        mm.then_inc(mm_sem, 1)

        @block.vector
        def _(vector):
            for t in range(RT):
                u = 2 * t
                vector.wait_ge(mm_sem, u + 1)
                nc.vector.tensor_reduce(
                    red_sb[:, t, :],
                    ps[:, :WIN].rearrange("p (o k) -> p o k", k=GRP),
                    axis=mybir.AxisListType.X,
                    op=mybir.AluOpType.max,
                ).then_inc(dve_sem, 1)

        @block.scalar
        def _(scalar):
            for t in range(RT):
                u = 2 * t + 1
                scalar.wait_ge(mm_sem, u + 1)
                nc.scalar.copy(cp_sb[:, t, :], ps[:, WIN:]).then_inc(act_sem, 1)

    nc.compile()
    return nc


def _get_nc():
    if "nc" not in _NC_CACHE:
        _NC_CACHE["nc"] = _build_nc()
    return _NC_CACHE["nc"]


# ---------------- host-side helpers (all float32, mirror reference) ----------


def _sample_descriptors(desc2, kp):
    """Bilinear sample of desc2 (B,C,H,W) at image-space (y,x) kp, L2-normed."""
    b, c, h, w = desc2.shape
    f = np.float32
    y = np.clip(kp[..., 0] / f(GS) - f(0.5), f(0.0), f(h - 1.0)).astype(f)
    x = np.clip(kp[..., 1] / f(GS) - f(0.5), f(0.0), f(w - 1.0)).astype(f)
    y0 = np.clip(np.floor(y), 0, h - 2).astype(np.int64)
    x0 = np.clip(np.floor(x), 0, w - 2).astype(np.int64)
    wy = (y - y0.astype(f))[..., None]
    wx = (x - x0.astype(f))[..., None]
    dmap = desc2.transpose(0, 2, 3, 1).reshape(b, h * w, c)

    def g(yi, xi):
        idx = yi * w + xi
        return np.take_along_axis(dmap, idx[..., None], axis=1)

    v = (
        g(y0, x0) * (1 - wy) * (1 - wx)
        + g(y0, x0 + 1) * (1 - wy) * wx
        + g(y0 + 1, x0) * wy * (1 - wx)
        + g(y0 + 1, x0 + 1) * wy * wx
    )
    n = np.sqrt(np.sum(v * v, axis=-1, keepdims=True)).astype(f)
    return (v / (n + f(1e-8))).astype(f)


def _nearest4(pts):
    """Flat ids (..., 4) of the 4 nearest grid-cell centers, matching the
    reference's top_k over all HW cells (ties -> lower flat id)."""
    f = np.float32
    y = pts[..., 0]
    x = pts[..., 1]
    cy = np.clip(np.floor(y / f(GS)).astype(np.int64), 0, H - 1)
    cx = np.clip(np.floor(x / f(GS)).astype(np.int64), 0, W - 1)
    by = np.clip(cy - 2, 0, H - 5)
    bx = np.clip(cx - 2, 0, W - 5)
    offs = np.arange(5, dtype=np.int64)
    iy = by[..., None] + offs          # (..., 5)
    ix = bx[..., None] + offs
    cyc = (f(GS) * iy + f(GS / 2.0)).astype(f)
    cxc = (f(GS) * ix + f(GS / 2.0)).astype(f)
    dy = y[..., None] - cyc
    dx = x[..., None] - cxc
    d2 = (dy * dy)[..., :, None] + (dx * dx)[..., None, :]   # (..., 5, 5)
    ids = iy[..., :, None] * W + ix[..., None, :]
    d2 = d2.reshape(d2.shape[:-2] + (25,))
    ids = ids.reshape(ids.shape[:-2] + (25,))
    order = np.argsort(d2, axis=-1, kind="stable")[..., :4]
    return np.take_along_axis(ids, order, axis=-1)


def _warp(p, Hm):
    f = np.float32
    xy = p[..., ::-1]
    ph = np.concatenate([xy, np.ones_like(xy[..., :1])], axis=-1)
    wp = np.einsum("bij,bmj->bmi", Hm, ph).astype(f)
    wp = wp[..., :2] / (wp[..., 2:3] + f(1e-8))
    return wp[..., ::-1].astype(f)


def _centers(ids):
    f = np.float32
    yy = (ids // W).astype(f) * f(GS) + f(GS / 2.0)
    xx = (ids % W).astype(f) * f(GS) + f(GS / 2.0)
    return np.stack([yy, xx], axis=-1)


def _smallest8_ids(sim):
    """Indices of the 8 smallest values per row of sim (B,N,N), reference
    tie-break (lower index wins)."""
    part = np.argpartition(sim, SOS_NEG + 1, axis=-1)[..., : SOS_NEG + 2]
    vals = np.take_along_axis(sim, part, axis=-1)
    order = np.lexsort((part, vals), axis=-1)[..., :SOS_NEG]
    return np.take_along_axis(part, order, axis=-1)


def kernel(kp1, w_kp1, kp1_desc, desc2, homo12):
    global LAST_RESULTS
    import os

    f = np.float32
    kp1 = np.asarray(kp1, f)
    w_kp1 = np.asarray(w_kp1, f)
    kp1_desc = np.asarray(kp1_desc, f)
    desc2 = np.asarray(desc2, f)
    homo12 = np.asarray(homo12, f)

    # ---------------- host geometry / small tensors ----------------
    w_kp1_desc = _sample_descriptors(desc2, w_kp1)                  # (B,N,C)
    pos = f(2.0) - f(2.0) * np.einsum("bnc,bnc->bn", kp1_desc, w_kp1_desc)

    cell4 = _nearest4(kp1)                                          # (B,N,4)
    kp1_cells = _centers(cell4.reshape(B, 4 * N))                   # (B,4N,2)
    warped = _warp(kp1_cells, homo12)                               # (B,4N,2)
    wcc = _nearest4(warped)                                         # (B,4N,4)
    ids16 = wcc.reshape(B, N, 16)                                   # neigh cells
    cell4_w = _nearest4(w_kp1)                                      # (B,N,4)

    eqk = cell4[:, :, :, None, None] == cell4[:, None, None, :, :]
    kp1_mask = eqk.sum(axis=(2, 4)).astype(f)                       # (B,N,N)
    eqw = ids16[:, :, :, None, None] == cell4_w[:, None, None, :, :]
    w_kp1_mask = eqw.sum(axis=(2, 4)).astype(f)                     # (B,N,N)

    # ---------------- sos (entirely host) ----------------
    k_sim = (f(2.0) - f(2.0) * np.einsum("bnc,bmc->bnm", kp1_desc, kp1_desc)
             + kp1_mask * f(5.0))
    w_sim = (f(2.0) - f(2.0) * np.einsum("bnc,bmc->bnm", w_kp1_desc, w_kp1_desc)
             + w_kp1_mask * f(5.0))
    k_ids = _smallest8_ids(k_sim)                                   # (B,N,8)
    w_ids = _smallest8_ids(w_sim)
    kd = np.take_along_axis(
        kp1_desc, k_ids.reshape(B, N * 8)[:, :, None], axis=1
    ).reshape(B, N, 8, C)
    wd = np.take_along_axis(
        w_kp1_desc, w_ids.reshape(B, N * 8)[:, :, None], axis=1
    ).reshape(B, N, 8, C)
    a = f(2.0) - f(2.0) * np.einsum("bnc,bnkc->bnk", kp1_desc, kd)
    bb = f(2.0) - f(2.0) * np.einsum("bnc,bnkc->bnk", w_kp1_desc, wd)
    sv = (a - bb).astype(f)
    sos = np.mean(np.sqrt(np.sum(sv * sv, axis=-1))).astype(f)

    # ---------------- device run: group-max candidates ----------------
    nc = _get_nc()
    desc2_flat = desc2.reshape(B, C, HW)
    # lhsT (p, t, i, n') = q[t*128+n', p+64i]
    lhsT_all = []
    for b in range(B):
        qT = kp1_desc[b].T.reshape(2, 64, RT, 128)     # (i, p, t, n')
        lhsT_all.append(
            np.ascontiguousarray(qT.transpose(1, 2, 0, 3)).astype(F8NP)
        )
    in_maps = []
    for b in range(B):
        for s in range(NSHARD):
            d = desc2_flat[b][:, s * SHW : (s + 1) * SHW]    # (C, SHW)
            rhs_b = np.ascontiguousarray(
                d.reshape(2, 64, SHW).transpose(1, 0, 2)
            ).astype(F8NP)                                    # (p, i, m)
            in_maps.append({"lhsT": lhsT_all[b], "rhs": rhs_b})

    want_trace = bool(int(os.environ.get("KT_TRACE", "0")))
    try:
        res = run_bass_kernel_spmd(
            nc, in_maps, core_ids=list(range(8)), trace=want_trace
        )
    except ModuleNotFoundError:
        res = run_bass_kernel_spmd(nc, in_maps, core_ids=list(range(8)), trace=False)
    LAST_RESULTS = res
    results = res.results

    # ---------------- host merge: top-K groups, exact rescore ------------
    # Vals[b, n, s, w, o]: group max (G=8 contiguous cells) in device precision
    Vals = np.empty((B, N, NSHARD, 2, NGRP), f)
    for ci in range(B * NSHARD):
        b, s = divmod(ci, NSHARD)
        c = np.asarray(results[ci]["cand"]).astype(f).reshape(128, RT, TCOLS)
        red = c[:, :, :NGRP]                                    # (128, RT, 256)
        raw = c[:, :, NGRP:].reshape(128, RT, NGRP, GRP).max(axis=3)
        for t in range(RT):
            Vals[b, t * 128 : (t + 1) * 128, s, 0] = red[:, t]
            Vals[b, t * 128 : (t + 1) * 128, s, 1] = raw[:, t]

    V = Vals.reshape(B, N, NSHARD * 2 * NGRP)                   # 2048 groups
    part = np.argpartition(-V, KSEL, axis=2)[:, :, :KSEL]       # (B, N, K)
    pv = np.take_along_axis(V, part, axis=2)
    vK = pv.min(axis=2)                                         # (B, N)

    # decode group id -> cell base: gid = ((s*2 + w)*NGRP + o)
    o = part % NGRP
    w = (part // NGRP) % 2
    s_ = part // (2 * NGRP)
    base = s_ * SHW + w * WIN + o * GRP                          # (B, N, K)
    cells = (base[..., None] + np.arange(GRP)).reshape(B, N, KSEL * GRP)

    hwdesc = desc2_flat.transpose(0, 2, 1)                      # (B, HW, C) f32
    gath = np.take_along_axis(
        hwdesc, cells.reshape(B, N * KSEL * GRP)[:, :, None], axis=1
    ).reshape(B, N, KSEL * GRP, C)
    ex = np.einsum("bnc,bnjc->bnj", kp1_desc, gath).astype(f)   # exact scores

    masked = (cells[..., None] == ids16[:, :, None, :]).any(axis=3)
    ex[masked] = -np.inf
    exs = -np.sort(-ex, axis=2)[:, :, :NUM_NEG]                 # (B, N, 16) desc
    t16 = exs[..., NUM_NEG - 1]

    repair = t16 < (vK + f(DELTA))                              # (B, N) bool
    if repair.any():
        rb, rn = np.nonzero(repair)
        rows = np.einsum("jc,jhc->jh", kp1_desc[rb, rn], hwdesc[rb])  # (R, HW)
        for j in range(len(rb)):
            rows[j, ids16[rb[j], rn[j]]] = -np.inf
        rs = -np.sort(-rows, axis=1)[:, :NUM_NEG]
        exs[rb, rn] = rs

    neg = f(2.0) - f(2.0) * exs                                 # (B, N, 16)
    fos = np.mean(
        np.maximum(pos[..., None] - neg + f(MARGIN), f(0.0)) ** 2
    ).astype(f)

    return np.asarray(fos + sos, dtype=np.float32)
